# revision 1
# baseline (speedup 1.0000x reference)
"""Trainium2 Bass kernel for nn_DA_conv: per-sample generated depthwise 3x3 conv
-> relu -> 1x1 pointwise conv (+bias) -> + x * channel_attention(altitude).

Data-parallel over batch: 8 samples -> 8 NeuronCores, weights replicated.

Per-core device pipeline:
  prologue:  feat = lrelu(W1 @ alt);  ktab[c,t] = <feat, W2[c*9+t,:]> (9 tiny matmuls)
             att = sigmoid(ca_w2 @ lrelu(ca_w1 @ alt))
             diag_t = diag(ktab[:,t]) built with one DVE tensor_scalar each
  main loop over half-chunks (16 image rows); the 9 depthwise taps run either
  on the TensorEngine (diagonal bf16 matmuls accumulating in PSUM) or, for
  DVE_HALVES, on the VectorEngine (bf16 2x-mode scalar_tensor_tensor chains;
  a one-element-shifted copy xb1 keeps the odd dx taps 4-byte aligned):
    PE  : 9 diagonal matmuls -> psum_s          (PE halves)
    DVE : tensor_scalar + 8 STT -> s_acc bf16   (DVE halves)
    ACT : relu -> srelu (bf16)
    PE  : pointwise conv_w.T @ srelu + diag(att_bf16) @ x_lo into psum_o
    ACT : osb = psum_o + conv_b
    DVE : osb += x_hi * att      (exact fp32 att as the STT scalar)
    DMA : osb -> DRAM  (per 512-column block)

x is sent as a bf16 hi/lo pair (same bytes as fp32, exact sum); the conv taps
use hi only, the residual uses hi (DVE, fp32 att) + lo (PE) so the large
x*att term is nearly exact. Host zero-pads x to [C, 130, 132] (1 row halo,
2+2 column pad) so every tap is a pure access-pattern offset.
"""

import os
from collections import deque
from contextlib import ExitStack

import ml_dtypes
import numpy as np

import concourse.bass as bass
import concourse.mybir as mybir
import concourse.tile as tile
from concourse import bacc
from concourse.bass_utils import run_bass_kernel_spmd

AF = mybir.ActivationFunctionType
ALU = mybir.AluOpType
F32 = mybir.dt.float32
F32R = mybir.dt.float32r
BF16 = mybir.dt.bfloat16

B, C, H, W = 8, 128, 128, 128
KK = 3
NT = KK * KK                 # 9 taps
HW = H * W
XOFF = 2                     # interior column offset in the padded layout
WP = W + 4                   # host-padded width (2 left, 2 right)
HP = H + 2                   # host-padded height (1 halo row each side)
R = 32                       # image rows per chunk
NCH = H // R                 # chunks
BR = 4                       # image rows per psum block (BR*W = 512 fp32 = 1 bank)
NBLK = R // BR               # psum blocks per chunk
HR = 16                      # image rows per half-chunk (scheduling unit)
BPH = HR // BR               # blocks per half-chunk
TAPS = [(dy, dx) for dy in (-1, 0, 1) for dx in (-1, 0, 1)]  # t = (dy+1)*3+(dx+1)
DVE_HALVES = frozenset()   # DVE tap path disabled (STT runs 1x-only: net loss)
TAIL_LAG = 2                 # blocks between taps and their consuming tail

last_results = None          # BassKernelResults of the most recent run


def _emit(tc, nc, d):
    ctx = d["ctx"]
    singles = ctx.enter_context(tc.tile_pool(name="singles", bufs=1))
    xpool = ctx.enter_context(tc.tile_pool(name="xpool", bufs=3))
    spool = ctx.enter_context(tc.tile_pool(name="spool", bufs=2))
    apool = ctx.enter_context(tc.tile_pool(name="apool", bufs=2))
    opool = ctx.enter_context(tc.tile_pool(name="opool", bufs=4))
    pss_pool = ctx.enter_context(tc.tile_pool(name="psum_s", bufs=4, space="PSUM"))
    pso_pool = ctx.enter_context(tc.tile_pool(name="psum_o", bufs=3, space="PSUM"))
    pm_pool = ctx.enter_context(tc.tile_pool(name="psum_m", bufs=1, space="PSUM"))

    def load(name, dram, shape, dt=F32):
        t = singles.tile(shape, dt, name=name, tag=name)
        nc.sync.dma_start(out=t, in_=dram)
        return t

    alt = load("alt_s", d["alt"], [128, 1])
    w1t = load("w1t_s", d["w1t"], [128, 128])
    w2t = load("w2t_s", d["w2t"], [128, C * NT])
    cwt = load("cwt_s", d["cwt"], [C, C], dt=BF16)
    cb = load("cb_s", d["cb"], [C, 1])
    ca1t = load("ca1t_s", d["ca1t"], [128, 16])
    ca2t = load("ca2t_s", d["ca2t"], [16, 128])
    iota = load("iota_s", d["iota"], [128, 128])
    cidx = load("cidx_s", d["cidx"], [128, 1])

    def leaky(name, psum_src, parts):
        """lrelu(v) = max(0.1*v, v), via ACT copy to SBUF then one DVE STT."""
        tmp = singles.tile([parts, 1], F32, name=f"{name}_t", tag=f"{name}_t")
        nc.scalar.activation(tmp, psum_src, AF.Copy)
        res = singles.tile([parts, 1], F32, name=name, tag=name)
        nc.vector.scalar_tensor_tensor(
            out=res, in0=tmp, scalar=0.1, in1=tmp, op0=ALU.mult, op1=ALU.max
        )
        return res

    # ---- kernel-generator MLP ----
    feat_ps = pm_pool.tile([128, 1], F32, name="feat_ps", tag="pm")
    nc.tensor.matmul(feat_ps, lhsT=w1t, rhs=alt, start=True, stop=True)
    feat = leaky("feat", feat_ps, 128)

    ktab_ps = pm_pool.tile([128, NT], F32, name="ktab_ps", tag="pm")
    w2r = w2t.rearrange("p (c t) -> p t c", t=NT)
    for t in range(NT):
        nc.tensor.matmul(
            ktab_ps[:, t : t + 1], lhsT=w2r[:, t, :], rhs=feat, start=True, stop=True
        )
    ktab = singles.tile([128, NT], F32, name="ktab", tag="ktab")
    nc.scalar.activation(ktab, ktab_ps, AF.Copy)

    # ---- channel attention ----
    a1_ps = pm_pool.tile([16, 1], F32, name="a1_ps", tag="pm")
    nc.tensor.matmul(a1_ps, lhsT=ca1t, rhs=alt, start=True, stop=True)
    a1 = leaky("a1", a1_ps, 16)
    att_ps = pm_pool.tile([128, 1], F32, name="att_ps", tag="pm")
    nc.tensor.matmul(att_ps, lhsT=ca2t, rhs=a1, start=True, stop=True)
    attv = singles.tile([128, 1], F32, name="attv", tag="attv")
    nc.scalar.activation(attv, att_ps, AF.Sigmoid)

    # ---- diagonal weight matrices ----
    diags = []
    for t in range(NT):
        dg = singles.tile([128, 128], BF16, name=f"diag{t}", tag=f"diag{t}")
        nc.vector.tensor_scalar(
            out=dg, in0=iota, scalar1=cidx, scalar2=ktab[:, t : t + 1],
            op0=ALU.is_equal, op1=ALU.mult,
        )
        diags.append(dg)
    attd = singles.tile([128, 128], BF16, name="attd", tag="attd")
    nc.vector.tensor_scalar(
        out=attd, in0=iota, scalar1=cidx, scalar2=attv,
        op0=ALU.is_equal, op1=ALU.mult,
    )

    x3h = d["xpad_hi"].rearrange("c (h w) -> c h w", w=WP)
    x3l = d["xpad_lo"].rearrange("c (h w) -> c h w", w=WP)
    out_d = d["out"]

    # ---- main loop over half-chunks, tails pipelined TAIL_LAG blocks late ----
    tails = deque()

    def flush(n):
        while len(tails) > n:
            tails.popleft()()

    for ci in range(NCH):
        y0 = ci * R
        xp = xpool.tile([128, R + 2, WP], BF16, name=f"xp{ci}", tag="xp")
        nc.sync.dma_start(out=xp, in_=x3h[:, y0 : y0 + R + 2, :])
        xpl = xpool.tile([128, R + 2, WP], BF16, name=f"xpl{ci}", tag="xpl")
        nc.sync.dma_start(out=xpl, in_=x3l[:, y0 : y0 + R + 2, :])
        xb1 = None
        if any((2 * ci + hh) in DVE_HALVES for hh in (0, 1)):
            # xb1[n] = xp_flat[n+1]: keeps dx=+-1 taps 4-byte aligned on DVE
            nflat = (R + 2) * WP
            xb1 = xpool.tile([128, nflat], BF16, name=f"xb1{ci}", tag="xb1")
            nc.vector.tensor_copy(
                out=xb1[:, 0 : nflat - 2],
                in_=xp.rearrange("p r c -> p (r c)")[:, 1 : nflat - 1],
            )
        srelu = spool.tile([128, R * W], BF16, name=f"sr{ci}", tag="sr")

        for h in (0, 1):
            u = 2 * ci + h
            hr0 = h * HR  # chunk-relative first image row of this half
            if u in DVE_HALVES:
                xb13 = xb1.rearrange("p (r c) -> p r c", c=WP)
                sacc = apool.tile([128, HR * W], BF16, name=f"sacc{u}", tag="sacc")
                for ti, (dy, dx) in enumerate(TAPS):
                    if dx == 0:
                        src = xp[:, 1 + hr0 + dy : 1 + hr0 + dy + HR, XOFF : XOFF + W]
                    elif dx == 1:
                        src = xb13[:, 1 + hr0 + dy : 1 + hr0 + dy + HR, XOFF : XOFF + W]
                    else:
                        src = xb13[:, 1 + hr0 + dy : 1 + hr0 + dy + HR, 0:W]
                    if ti == 0:
                        nc.vector.tensor_scalar_mul(
                            out=sacc, in0=src, scalar1=ktab[:, ti : ti + 1]
                        )
                    else:
                        nc.vector.scalar_tensor_tensor(
                            out=sacc, in0=src, scalar=ktab[:, ti : ti + 1],
                            in1=sacc, op0=ALU.mult, op1=ALU.add,
                        )
                sl_h = slice(hr0 * W, (hr0 + HR) * W)
                nc.scalar.activation(srelu[:, sl_h], sacc, AF.Relu)
                for bb in range(BPH):
                    r0 = hr0 + bb * BR
                    tails.append(_make_tail(nc, pso_pool, opool, xp, xpl, srelu,
                                            None, cwt, attd, attv, cb, out_d,
                                            ci, r0, y0))
                    flush(TAIL_LAG)
            else:
                for bb in range(BPH):
                    r0 = hr0 + bb * BR
                    pss = pss_pool.tile([128, BR * W], F32, name=f"pss{u}_{bb}",
                                        tag="pss")
                    for ti, (dy, dx) in enumerate(TAPS):
                        rhs = xp[:, 1 + r0 + dy : 1 + r0 + dy + BR,
                                 XOFF + dx : XOFF + dx + W]
                        nc.tensor.matmul(
                            pss, lhsT=diags[ti], rhs=rhs,
                            start=(ti == 0), stop=(ti == NT - 1),
                        )
                    tails.append(_make_tail(nc, pso_pool, opool, xp, xpl, srelu,
                                            pss, cwt, attd, attv, cb, out_d,
                                            ci, r0, y0))
                    flush(TAIL_LAG)
    flush(0)


def _make_tail(nc, pso_pool, opool, xp, xpl, srelu, pss, cwt, attd, attv, cb,
               out_d, ci, r0, y0):
    """relu (PE halves) + pointwise + lo-residual + biased evac + hi-residual +
    store for the block at chunk-relative rows [r0, r0+BR)."""

    def tail():
        sl = slice(r0 * W, (r0 + BR) * W)
        if pss is not None:
            nc.scalar.activation(srelu[:, sl], pss, AF.Relu)
        pso = pso_pool.tile([128, BR * W], F32, name=f"pso{ci}_{r0}", tag="pso")
        nc.tensor.matmul(pso, lhsT=cwt, rhs=srelu[:, sl], start=True, stop=True)
        osb = opool.tile([128, BR * W], F32, name=f"ob{ci}_{r0}", tag="ob")
        nc.scalar.activation(osb, pso, AF.Identity, bias=cb)
        nc.vector.scalar_tensor_tensor(
            out=osb, in0=xp[:, 1 + r0 : 1 + r0 + BR, XOFF : XOFF + W],
            scalar=attv, in1=osb, op0=ALU.mult, op1=ALU.add,
        )
        nc.vector.scalar_tensor_tensor(
            out=osb, in0=xpl[:, 1 + r0 : 1 + r0 + BR, XOFF : XOFF + W],
            scalar=attv, in1=osb, op0=ALU.mult, op1=ALU.add,
        )
        nc.sync.dma_start(out=out_d[:, (y0 + r0) * W : (y0 + r0 + BR) * W], in_=osb)

    return tail


def build_module():
    nc = bacc.Bacc(
        "TRN2",
        target_bir_lowering=False,
        debug=False,
        enable_asserts=False,
        num_devices=B,
    )
    d = {
        "xpad_hi": nc.dram_tensor("xpad_hi", [C, HP * WP], BF16, kind="ExternalInput").ap(),
        "xpad_lo": nc.dram_tensor("xpad_lo", [C, HP * WP], BF16, kind="ExternalInput").ap(),
        "alt": nc.dram_tensor("alt", [128, 1], F32, kind="ExternalInput").ap(),
        "w1t": nc.dram_tensor("w1t", [128, 128], F32, kind="ExternalInput").ap(),
        "w2t": nc.dram_tensor("w2t", [128, C * NT], F32, kind="ExternalInput").ap(),
        "cwt": nc.dram_tensor("cwt", [C, C], BF16, kind="ExternalInput").ap(),
        "cb": nc.dram_tensor("cb", [C, 1], F32, kind="ExternalInput").ap(),
        "ca1t": nc.dram_tensor("ca1t", [128, 16], F32, kind="ExternalInput").ap(),
        "ca2t": nc.dram_tensor("ca2t", [16, 128], F32, kind="ExternalInput").ap(),
        "iota": nc.dram_tensor("iota", [128, 128], F32, kind="ExternalInput").ap(),
        "cidx": nc.dram_tensor("cidx", [128, 1], F32, kind="ExternalInput").ap(),
        "out": nc.dram_tensor("out", [C, HW], F32, kind="ExternalOutput").ap(),
    }
    with tile.TileContext(nc) as tc:
        with ExitStack() as ctx:
            d["ctx"] = ctx
            _emit(tc, nc, d)
    nc.finalize()
    return nc


_module_cache = None


def _get_module():
    global _module_cache
    if _module_cache is None:
        _module_cache = build_module()
    return _module_cache


def make_in_maps(x, altitude, W1, W2, conv_w, conv_b, ca_w1, ca_w2):
    f = np.float32
    x = np.asarray(x, dtype=f)
    altitude = np.asarray(altitude, dtype=f)
    xpad = np.zeros((B, C, HP, WP), dtype=f)
    xpad[:, :, 1 : H + 1, XOFF : XOFF + W] = x
    xhi_f = xpad.astype(ml_dtypes.bfloat16)
    xlo = np.ascontiguousarray(
        (xpad - xhi_f.astype(f)).astype(ml_dtypes.bfloat16).reshape(B, C, HP * WP)
    )
    xhi = np.ascontiguousarray(xhi_f.reshape(B, C, HP * WP))
    shared = {
        "w1t": np.ascontiguousarray(np.asarray(W1, dtype=f).T),
        "w2t": np.ascontiguousarray(np.asarray(W2, dtype=f).T),
        "cwt": np.ascontiguousarray(
            np.asarray(conv_w, dtype=f).T.astype(ml_dtypes.bfloat16)
        ),
        "cb": np.ascontiguousarray(np.asarray(conv_b, dtype=f).reshape(C, 1)),
        "ca1t": np.ascontiguousarray(np.asarray(ca_w1, dtype=f).T),
        "ca2t": np.ascontiguousarray(np.asarray(ca_w2, dtype=f).T),
        "iota": np.ascontiguousarray(
            np.broadcast_to(np.arange(128, dtype=f), (128, 128))
        ),
        "cidx": np.arange(128, dtype=f).reshape(128, 1).copy(),
    }
    return [
        dict(shared, xpad_hi=xhi[bb], xpad_lo=xlo[bb],
             alt=np.ascontiguousarray(altitude[bb].reshape(128, 1)))
        for bb in range(B)
    ]


def kernel(x, altitude, W1, W2, conv_w, conv_b, ca_w1, ca_w2):
    global last_results
    in_maps = make_in_maps(x, altitude, W1, W2, conv_w, conv_b, ca_w1, ca_w2)
    nc = _get_module()
    trace = os.environ.get("KERNEL_TRACE", "0") == "1"
    last_results = run_bass_kernel_spmd(
        nc, in_maps, core_ids=list(range(B)), trace=trace
    )
    out = np.stack(
        [last_results.results[bb]["out"].reshape(C, H, W) for bb in range(B)]
    )
    return out



# revision 4
# speedup vs baseline: 1.1551x; 1.1551x over previous
"""Trainium2 Bass kernel for nn_DA_conv: per-sample generated depthwise 3x3 conv
-> relu -> 1x1 pointwise conv (+bias) -> + x * channel_attention(altitude).

Data-parallel over batch: 8 samples -> 8 NeuronCores, weights replicated.

v2 design (from trace analysis of the 100.4us baseline):
  * PE is the bottleneck (10 column-passes per output: 9 depthwise taps + 1
    pointwise). Depthwise work is split between the TensorEngine (diag bf16
    matmuls, rows [0, PE_ROWS) of each chunk) and the VectorEngine (rows
    [PE_ROWS, R): per-tap tensor_scalar_mul at 4x bf16 + tensor_tensor add at
    2x bf16, with a one-element-shifted copy xb1 keeping odd-dx taps aligned).
  * The residual x*att moved from DVE (2 STT/block, 48us) to the PE: one
    diag(att) matmul accumulated into the pointwise PSUM group (+7us PE,
    -47us DVE, kills the serial DVE drain tail).
  * x is sent bf16-only (no hi/lo pair): halves input DMA; rel err ~5e-3
    vs the 2e-2 gate (host-simulated).
  * Small chunks (R=16 rows) so the first depthwise matmul starts ~3.5us in
    (the baseline idled the PE for ~19us waiting on a 2.2MB chunk-0 DMA).
  * Diagonal weight tiles built with ONE tensor_tensor mult against a
    host-provided replicated-identity mask (baseline: 10 serial DVE ops).

Per-core engine budget at PE_ROWS=10: PE ~54us, DVE ~53us, ACT ~38us,
DMA ~13MB (~36us), all overlapped -> target span ~58us.
"""

import os
from collections import deque
from contextlib import ExitStack

import ml_dtypes
import numpy as np

import concourse.bass as bass
import concourse.mybir as mybir
import concourse.tile as tile
from concourse import bacc
from concourse.bass_utils import run_bass_kernel_spmd

AF = mybir.ActivationFunctionType
ALU = mybir.AluOpType
F32 = mybir.dt.float32
BF16 = mybir.dt.bfloat16

B, C, H, W = 8, 128, 128, 128
KK = 3
NT = KK * KK                 # 9 taps
HW = H * W
XOFF = 2                     # interior column offset in the padded layout
WP = W + 4                   # host-padded width (2 left, 2 right)
HP = H + 2                   # host-padded height (1 halo row each side)
R = 16                       # image rows per chunk
NCH = H // R                 # 8 chunks
PE_ROWS = 10                 # chunk rows [0, PE_ROWS) -> TensorE depthwise
DVE_ROWS = R - PE_ROWS       # chunk rows [PE_ROWS, R) -> VectorE depthwise
TAIL_ROWS = 4                # rows per pointwise/residual/store unit (1 PSUM bank)
TAPS = [(dy, dx) for dy in (-1, 0, 1) for dx in (-1, 0, 1)]  # t = (dy+1)*3+(dx+1)
TAIL_LAG = 3                 # tail units kept pending (pipelining depth)

last_results = None          # BassKernelResults of the most recent run


def _pe_blocks():
    """Split [0, PE_ROWS) into sub-blocks of <=4 rows (<=512 fp32 = 1 PSUM bank)."""
    blocks = []
    r = 0
    while r < PE_ROWS:
        rr = min(4, PE_ROWS - r)
        blocks.append((r, r + rr))
        r += rr
    return blocks


def _emit(tc, nc, d):
    ctx = d["ctx"]
    singles = ctx.enter_context(tc.tile_pool(name="singles", bufs=1))
    xpool = ctx.enter_context(tc.tile_pool(name="xpool", bufs=3))
    xbpool = ctx.enter_context(tc.tile_pool(name="xbpool", bufs=2))
    spool = ctx.enter_context(tc.tile_pool(name="spool", bufs=2))
    apool = ctx.enter_context(tc.tile_pool(name="apool", bufs=2))
    tpool = ctx.enter_context(tc.tile_pool(name="tpool", bufs=2))
    opool = ctx.enter_context(tc.tile_pool(name="opool", bufs=3))
    pss_pool = ctx.enter_context(tc.tile_pool(name="psum_s", bufs=4, space="PSUM"))
    pso_pool = ctx.enter_context(tc.tile_pool(name="psum_o", bufs=3, space="PSUM"))

    def load(name, dram, shape, dt=F32):
        t = singles.tile(shape, dt, name=name, tag=name)
        nc.sync.dma_start(out=t, in_=dram)
        return t

    alt = load("alt_s", d["alt"], [128, 1])
    w1t = load("w1t_s", d["w1t"], [128, 128])
    w2t = load("w2t_s", d["w2t"], [128, C * NT], dt=BF16)
    cwt = load("cwt_s", d["cwt"], [C, C], dt=BF16)
    cb = load("cb_s", d["cb"], [C, 1])
    ca1t = load("ca1t_s", d["ca1t"], [128, 16])
    ca2t = load("ca2t_s", d["ca2t"], [16, 128])
    mask = load("mask_s", d["mask"], [128, NT * 128], dt=BF16)

    def leaky(name, psum_src, parts, dt=F32):
        """lrelu(v) = max(0.1*v, v), via ACT copy to SBUF then one DVE STT."""
        tmp = singles.tile([parts, 1], F32, name=f"{name}_t", tag=f"{name}_t")
        nc.scalar.activation(tmp, psum_src, AF.Copy)
        res = singles.tile([parts, 1], dt, name=name, tag=name)
        nc.vector.scalar_tensor_tensor(
            out=res, in0=tmp, scalar=0.1, in1=tmp, op0=ALU.mult, op1=ALU.max
        )
        return res

    # ---- kernel-generator MLP ----
    feat_ps = pss_pool.tile([128, 1], F32, name="feat_ps", tag="pss")
    nc.tensor.matmul(feat_ps, lhsT=w1t, rhs=alt, start=True, stop=True)
    feat = leaky("feat", feat_ps, 128, dt=BF16)

    ktab_ps = pss_pool.tile([128, NT], F32, name="ktab_ps", tag="pss")
    w2r = w2t.rearrange("p (c t) -> p t c", t=NT)
    for t in range(NT):
        nc.tensor.matmul(
            ktab_ps[:, t : t + 1], lhsT=w2r[:, t, :], rhs=feat, start=True, stop=True
        )
    ktab = singles.tile([128, NT], F32, name="ktab", tag="ktab")
    nc.scalar.activation(ktab, ktab_ps, AF.Copy)

    # ---- channel attention ----
    a1_ps = pss_pool.tile([16, 1], F32, name="a1_ps", tag="pss")
    nc.tensor.matmul(a1_ps, lhsT=ca1t, rhs=alt, start=True, stop=True)
    a1 = leaky("a1", a1_ps, 16)
    att_ps = pss_pool.tile([128, 1], F32, name="att_ps", tag="pss")
    nc.tensor.matmul(att_ps, lhsT=ca2t, rhs=a1, start=True, stop=True)
    attv = singles.tile([128, 1], F32, name="attv", tag="attv")
    nc.scalar.activation(attv, att_ps, AF.Sigmoid)

    # ---- diagonal weight matrices: dg_all[:, t*128+j] = I[p,j] * ktab[p,t] ----
    dg_all = singles.tile([128, NT * 128], BF16, name="dg_all", tag="dg_all")
    ktab_b = ktab.unsqueeze(2).broadcast_to([128, NT, 128])
    nc.vector.tensor_tensor(
        out=dg_all.rearrange("p (t c) -> p t c", t=NT),
        in0=mask.rearrange("p (t c) -> p t c", t=NT),
        in1=ktab_b, op=ALU.mult,
    )
    attd = singles.tile([128, 128], BF16, name="attd", tag="attd")
    attv_b = attv.broadcast_to([128, 128])
    nc.vector.tensor_tensor(out=attd, in0=mask[:, 0:128], in1=attv_b, op=ALU.mult)

    x3 = d["xpad"].rearrange("c (h w) -> c h w", w=WP)
    out_d = d["out"]
    NB1 = (DVE_ROWS + 2) * WP    # xb1 flat length (rows needed by DVE taps + halo)

    # ---- main loop over chunks, tails pipelined TAIL_LAG units late ----
    tails = deque()

    def flush(n):
        while len(tails) > n:
            tails.popleft()()

    for ci in range(NCH):
        y0 = ci * R
        xp = xpool.tile([128, R + 2, WP], BF16, name=f"xp{ci}", tag="xp")
        nc.sync.dma_start(out=xp, in_=x3[:, y0 : y0 + R + 2, :])
        xpf = xp.rearrange("p r c -> p (r c)")

        # xb1[i] = xp_flat[PE_ROWS*WP + 1 + i]: one-element shift keeps the
        # odd-dx DVE taps 4-byte aligned (2x/4x DVE modes need it)
        xb1 = xbpool.tile([128, NB1], BF16, name=f"xb1{ci}", tag="xb1")
        nc.vector.tensor_copy(
            out=xb1[:, 0 : NB1 - 2],
            in_=xpf[:, PE_ROWS * WP + 1 : PE_ROWS * WP + NB1 - 1],
        )
        xb13 = xb1.rearrange("p (r c) -> p r c", c=WP)

        srelu = spool.tile([128, R * W], BF16, name=f"sr{ci}", tag="sr")

        # -- DVE depthwise: rows [PE_ROWS, R) --
        sacc = apool.tile([128, DVE_ROWS * W], BF16, name=f"sacc{ci}", tag="sacc")
        for ti, (dy, dx) in enumerate(TAPS):
            if dx == 0:
                src = xp[:, 1 + PE_ROWS + dy : 1 + PE_ROWS + DVE_ROWS + dy,
                         XOFF : XOFF + W]
            elif dx == 1:
                src = xb13[:, 1 + dy : 1 + DVE_ROWS + dy, XOFF : XOFF + W]
            else:
                src = xb13[:, 1 + dy : 1 + DVE_ROWS + dy, 0:W]
            if ti == 0:
                nc.vector.tensor_scalar_mul(
                    out=sacc, in0=src, scalar1=ktab[:, ti : ti + 1]
                )
            else:
                tmp = tpool.tile([128, DVE_ROWS * W], BF16, name=f"tp{ci}_{ti}",
                                 tag="tp")
                nc.vector.tensor_scalar_mul(
                    out=tmp, in0=src, scalar1=ktab[:, ti : ti + 1]
                )
                nc.vector.tensor_tensor(out=sacc, in0=tmp, in1=sacc, op=ALU.add)
        nc.scalar.activation(srelu[:, PE_ROWS * W : R * W], sacc, AF.Relu)

        # -- PE depthwise: rows [0, PE_ROWS) in <=4-row PSUM blocks --
        for rs, re in _pe_blocks():
            rows = re - rs
            pss = pss_pool.tile([128, rows * W], F32, name=f"pss{ci}_{rs}",
                                tag="pss")
            for ti, (dy, dx) in enumerate(TAPS):
                rhs = xp[:, 1 + rs + dy : 1 + re + dy, XOFF + dx : XOFF + dx + W]
                nc.tensor.matmul(
                    pss, lhsT=dg_all[:, ti * 128 : (ti + 1) * 128], rhs=rhs,
                    start=(ti == 0), stop=(ti == NT - 1),
                )
            nc.scalar.activation(srelu[:, rs * W : re * W], pss, AF.Relu)

        # -- tails: pointwise + residual + biased evac + store --
        for tr in range(0, R, TAIL_ROWS):
            tails.append(_make_tail(nc, pso_pool, opool, xp, srelu, cwt, attd,
                                    cb, out_d, ci, tr, y0))
            flush(TAIL_LAG)
    flush(0)


def _make_tail(nc, pso_pool, opool, xp, srelu, cwt, attd, cb, out_d, ci, tr, y0):
    """pointwise conv + diag(att)@x residual into one PSUM group, then biased
    ACT evac and DMA out, for chunk-relative rows [tr, tr+TAIL_ROWS)."""

    def tail():
        sl = slice(tr * W, (tr + TAIL_ROWS) * W)
        pso = pso_pool.tile([128, TAIL_ROWS * W], F32, name=f"pso{ci}_{tr}",
                            tag="pso")
        nc.tensor.matmul(pso, lhsT=cwt, rhs=srelu[:, sl], start=True, stop=False)
        nc.tensor.matmul(
            pso, lhsT=attd,
            rhs=xp[:, 1 + tr : 1 + tr + TAIL_ROWS, XOFF : XOFF + W],
            start=False, stop=True,
        )
        osb = opool.tile([128, TAIL_ROWS * W], F32, name=f"ob{ci}_{tr}", tag="ob")
        nc.scalar.activation(osb, pso, AF.Identity, bias=cb)
        nc.sync.dma_start(
            out=out_d[:, (y0 + tr) * W : (y0 + tr + TAIL_ROWS) * W], in_=osb
        )

    return tail


def build_module():
    nc = bacc.Bacc(
        "TRN2",
        target_bir_lowering=False,
        debug=False,
        enable_asserts=False,
        num_devices=B,
    )
    d = {
        "xpad": nc.dram_tensor("xpad", [C, HP * WP], BF16, kind="ExternalInput").ap(),
        "alt": nc.dram_tensor("alt", [128, 1], F32, kind="ExternalInput").ap(),
        "w1t": nc.dram_tensor("w1t", [128, 128], F32, kind="ExternalInput").ap(),
        "w2t": nc.dram_tensor("w2t", [128, C * NT], BF16, kind="ExternalInput").ap(),
        "cwt": nc.dram_tensor("cwt", [C, C], BF16, kind="ExternalInput").ap(),
        "cb": nc.dram_tensor("cb", [C, 1], F32, kind="ExternalInput").ap(),
        "ca1t": nc.dram_tensor("ca1t", [128, 16], F32, kind="ExternalInput").ap(),
        "ca2t": nc.dram_tensor("ca2t", [16, 128], F32, kind="ExternalInput").ap(),
        "mask": nc.dram_tensor("mask", [128, NT * 128], BF16, kind="ExternalInput").ap(),
        "out": nc.dram_tensor("out", [C, HW], F32, kind="ExternalOutput").ap(),
    }
    with tile.TileContext(nc) as tc:
        with ExitStack() as ctx:
            d["ctx"] = ctx
            _emit(tc, nc, d)
    nc.finalize()
    return nc


_module_cache = None


def _get_module():
    global _module_cache
    if _module_cache is None:
        _module_cache = build_module()
    return _module_cache


def make_in_maps(x, altitude, W1, W2, conv_w, conv_b, ca_w1, ca_w2):
    f = np.float32
    bf = ml_dtypes.bfloat16
    x = np.asarray(x, dtype=f)
    altitude = np.asarray(altitude, dtype=f)
    xpad = np.zeros((B, C, HP, WP), dtype=f)
    xpad[:, :, 1 : H + 1, XOFF : XOFF + W] = x
    xq = np.ascontiguousarray(xpad.astype(bf).reshape(B, C, HP * WP))
    shared = {
        "w1t": np.ascontiguousarray(np.asarray(W1, dtype=f).T),
        "w2t": np.ascontiguousarray(np.asarray(W2, dtype=f).T.astype(bf)),
        "cwt": np.ascontiguousarray(np.asarray(conv_w, dtype=f).T.astype(bf)),
        "cb": np.ascontiguousarray(np.asarray(conv_b, dtype=f).reshape(C, 1)),
        "ca1t": np.ascontiguousarray(np.asarray(ca_w1, dtype=f).T),
        "ca2t": np.ascontiguousarray(np.asarray(ca_w2, dtype=f).T),
        "mask": np.ascontiguousarray(
            np.tile(np.eye(128, dtype=f), (1, NT)).astype(bf)
        ),
    }
    return [
        dict(shared, xpad=xq[bb],
             alt=np.ascontiguousarray(altitude[bb].reshape(128, 1)))
        for bb in range(B)
    ]


def kernel(x, altitude, W1, W2, conv_w, conv_b, ca_w1, ca_w2):
    global last_results
    in_maps = make_in_maps(x, altitude, W1, W2, conv_w, conv_b, ca_w1, ca_w2)
    nc = _get_module()
    trace = os.environ.get("KERNEL_TRACE", "0") == "1"
    last_results = run_bass_kernel_spmd(
        nc, in_maps, core_ids=list(range(B)), trace=trace
    )
    out = np.stack(
        [last_results.results[bb]["out"].reshape(C, H, W) for bb in range(B)]
    )
    return out


# revision 5
# speedup vs baseline: 1.2064x; 1.0445x over previous
"""Trainium2 Bass kernel for nn_DA_conv: per-sample generated depthwise 3x3 conv
-> relu -> 1x1 pointwise conv (+bias) -> + x * channel_attention(altitude).

Data-parallel over batch: 8 samples -> 8 NeuronCores, weights replicated.

v3 design (trace-driven, from the 86.9us v2):
  * Depthwise split: TensorE does chunk rows [0, PE_ROWS) as diag bf16 matmuls
    (~0.48us/row), VectorE does rows [PE_ROWS, R) as tensor_scalar_mul (2x bf16)
    + depth-4 tree of tensor_tensor adds (2x bf16) (~1.4us/row). Balanced at
    PE_ROWS=22 of R=32.
  * Residual x*att rides the pointwise PSUM group as a diag(att) matmul.
  * R=32 chunks: DVE per-op overhead (~150ns x 17 ops/chunk) amortizes better.
  * xb1 (one-element-shifted x for odd-dx DVE tap alignment) comes from a
    second DMA of the same padded dram tensor at +1 element offset -- no DVE
    copy.
  * All 8 weight tensors merged into 2 blob DMAs (DMA issue costs ~600ns of
    serial Sync-engine time each); issue order: f32 blob, bf16 blob, chunk-0 x.
    First depthwise matmul starts ~5us in.
  * Output stored bf16 (halves store DMA, trims ACT evac); host upcasts.
    Host-simulated rel err ~8e-3 vs the 2e-2 gate.
"""

import os
from collections import deque
from contextlib import ExitStack

import ml_dtypes
import numpy as np

import concourse.bass as bass
import concourse.mybir as mybir
import concourse.tile as tile
from concourse import bacc
from concourse.bass_utils import run_bass_kernel_spmd

AF = mybir.ActivationFunctionType
ALU = mybir.AluOpType
F32 = mybir.dt.float32
BF16 = mybir.dt.bfloat16

B, C, H, W = 8, 128, 128, 128
KK = 3
NT = KK * KK                 # 9 taps
HW = H * W
XOFF = 2                     # interior column offset in the padded layout
WP = W + 4                   # host-padded width (2 left, 2 right)
HP = H + 2                   # host-padded height (1 halo row each side)
R = 32                       # image rows per chunk
NCH = H // R                 # 4 chunks
PE_ROWS = 22                 # chunk rows [0, PE_ROWS) -> TensorE depthwise
DVE_ROWS = R - PE_ROWS       # chunk rows [PE_ROWS, R) -> VectorE depthwise
TAPS = [(dy, dx) for dy in (-1, 0, 1) for dx in (-1, 0, 1)]  # t = (dy+1)*3+(dx+1)
TAIL_LAG = 2                 # 8-row tail units kept pending (pipelining depth)

# f32 blob column layout: w1t | alt | cb | ca1t | ca2t
F_W1T, F_ALT, F_CB, F_CA1T, F_CA2T = 0, 128, 129, 130, 146
F_COLS = 146 + 128
# bf16 blob column layout: w2t | cwt | mask
B_W2T, B_CWT, B_MASK = 0, NT * 128, NT * 128 + 128
B_COLS = NT * 128 + 128 + NT * 128

last_results = None          # BassKernelResults of the most recent run


def _pe_blocks():
    """Split [0, PE_ROWS) into sub-blocks of <=4 rows (<=512 fp32 = 1 PSUM bank)."""
    blocks = []
    r = 0
    while r < PE_ROWS:
        rr = min(4, PE_ROWS - r)
        blocks.append((r, r + rr))
        r += rr
    return blocks


def _emit(tc, nc, d):
    ctx = d["ctx"]
    singles = ctx.enter_context(tc.tile_pool(name="singles", bufs=1))
    xpool = ctx.enter_context(tc.tile_pool(name="xpool", bufs=3))
    xbpool = ctx.enter_context(tc.tile_pool(name="xbpool", bufs=2))
    spool = ctx.enter_context(tc.tile_pool(name="spool", bufs=2))
    tpool = ctx.enter_context(tc.tile_pool(name="tpool", bufs=12))
    opool = ctx.enter_context(tc.tile_pool(name="opool", bufs=3))
    pss_pool = ctx.enter_context(tc.tile_pool(name="psum_s", bufs=4, space="PSUM"))
    pso_pool = ctx.enter_context(tc.tile_pool(name="psum_o", bufs=3, space="PSUM"))

    fblob = singles.tile([128, F_COLS], F32, name="fblob", tag="fblob")
    nc.sync.dma_start(out=fblob, in_=d["fblob"])
    bblob = singles.tile([128, B_COLS], BF16, name="bblob", tag="bblob")
    nc.sync.dma_start(out=bblob, in_=d["bblob"])

    alt = fblob[:, F_ALT : F_ALT + 1]
    w1t = fblob[:, F_W1T : F_W1T + 128]
    cb = fblob[:, F_CB : F_CB + 1]
    ca1t = fblob[:, F_CA1T : F_CA1T + 16]
    ca2t = fblob[0:16, F_CA2T : F_CA2T + 128]
    w2t = bblob[:, B_W2T : B_W2T + NT * 128]
    cwt = bblob[:, B_CWT : B_CWT + 128]
    mask = bblob[:, B_MASK : B_MASK + NT * 128]

    # x DMAs for the first chunks go out right after the weight blobs so the
    # transfer overlaps the prologue math (issue order == Sync queue order).
    x3 = d["xpad"].rearrange("c (h w) -> c h w", w=WP)
    xpf_d = d["xpad"]
    NB1 = (DVE_ROWS + 2) * WP    # xb1 flat length (DVE tap rows + dy halo)
    xps, xb1s = [], []
    for ci in range(NCH):
        y0 = ci * R
        xp = xpool.tile([128, R + 2, WP], BF16, name=f"xp{ci}", tag="xp")
        nc.sync.dma_start(out=xp, in_=x3[:, y0 : y0 + R + 2, :])
        xb1 = xbpool.tile([128, NB1], BF16, name=f"xb1{ci}", tag="xb1")
        base = (y0 + PE_ROWS) * WP
        nc.sync.dma_start(
            out=xb1[:, 0 : NB1 - 2], in_=xpf_d[:, base + 1 : base + NB1 - 1]
        )
        xps.append(xp)
        xb1s.append(xb1)
        if ci == 0:
            _emit_prologue(tc, nc, d, singles, pss_pool,
                           alt, w1t, cb, ca1t, ca2t, w2t, mask)

    ktab = d["ktab"]
    dg_all = d["dg_all"]
    attd = d["attd"]
    out_d = d["out"]

    tails = deque()

    def flush(n):
        while len(tails) > n:
            tails.popleft()()

    for ci in range(NCH):
        y0 = ci * R
        xp = xps[ci]
        xb13 = xb1s[ci].rearrange("p (r c) -> p r c", c=WP)
        srelu = spool.tile([128, R * W], BF16, name=f"sr{ci}", tag="sr")

        # -- DVE depthwise: rows [PE_ROWS, R): 9 products, depth-4 add tree --
        prods = []
        for ti, (dy, dx) in enumerate(TAPS):
            if dx == 0:
                src = xp[:, 1 + PE_ROWS + dy : 1 + PE_ROWS + DVE_ROWS + dy,
                         XOFF : XOFF + W]
            elif dx == 1:
                src = xb13[:, 1 + dy : 1 + DVE_ROWS + dy, XOFF : XOFF + W]
            else:
                src = xb13[:, 1 + dy : 1 + DVE_ROWS + dy, 0:W]
            t = tpool.tile([128, DVE_ROWS * W], BF16, name=f"tp{ci}_{ti}", tag="tp")
            nc.vector.tensor_scalar_mul(out=t, in0=src, scalar1=ktab[:, ti : ti + 1])
            prods.append(t)
        while len(prods) > 1:
            nxt = []
            for i in range(0, len(prods) - 1, 2):
                a, b = prods[i], prods[i + 1]
                nc.vector.tensor_tensor(out=a, in0=b, in1=a, op=ALU.add)
                nxt.append(a)
            if len(prods) % 2:
                nxt.append(prods[-1])
            prods = nxt
        nc.scalar.activation(srelu[:, PE_ROWS * W : R * W], prods[0], AF.Relu)

        # -- PE depthwise: rows [0, PE_ROWS) in <=4-row PSUM blocks --
        for rs, re in _pe_blocks():
            rows = re - rs
            pss = pss_pool.tile([128, rows * W], F32, name=f"pss{ci}_{rs}",
                                tag="pss")
            for ti, (dy, dx) in enumerate(TAPS):
                rhs = xp[:, 1 + rs + dy : 1 + re + dy, XOFF + dx : XOFF + dx + W]
                nc.tensor.matmul(
                    pss, lhsT=dg_all[:, ti * 128 : (ti + 1) * 128], rhs=rhs,
                    start=(ti == 0), stop=(ti == NT - 1),
                )
            nc.scalar.activation(srelu[:, rs * W : re * W], pss, AF.Relu)

        # -- tails: 8-row units (2 PSUM banks of pointwise+residual, 1 store) --
        for tr in range(0, R, 8):
            tails.append(_make_tail(nc, pso_pool, opool, xp, srelu, cwt, attd,
                                    cb, out_d, ci, tr, y0))
            flush(TAIL_LAG)
    flush(0)


def _emit_prologue(tc, nc, d, singles, pss_pool, alt, w1t, cb, ca1t, ca2t,
                   w2t, mask):
    def leaky(name, psum_src, parts, dt=F32):
        """lrelu(v) = max(0.1*v, v), via ACT copy to SBUF then one DVE STT."""
        tmp = singles.tile([parts, 1], F32, name=f"{name}_t", tag=f"{name}_t")
        nc.scalar.activation(tmp, psum_src, AF.Copy)
        res = singles.tile([parts, 1], dt, name=name, tag=name)
        nc.vector.scalar_tensor_tensor(
            out=res, in0=tmp, scalar=0.1, in1=tmp, op0=ALU.mult, op1=ALU.max
        )
        return res

    # ---- kernel-generator MLP ----
    feat_ps = pss_pool.tile([128, 1], F32, name="feat_ps", tag="pss")
    nc.tensor.matmul(feat_ps, lhsT=w1t, rhs=alt, start=True, stop=True)
    feat = leaky("feat", feat_ps, 128, dt=BF16)

    ktab_ps = pss_pool.tile([128, NT], F32, name="ktab_ps", tag="pss")
    w2r = w2t.rearrange("p (c t) -> p t c", t=NT)
    for t in range(NT):
        nc.tensor.matmul(
            ktab_ps[:, t : t + 1], lhsT=w2r[:, t, :], rhs=feat, start=True, stop=True
        )
    ktab = singles.tile([128, NT], F32, name="ktab", tag="ktab")
    nc.scalar.activation(ktab, ktab_ps, AF.Copy)

    # ---- channel attention ----
    a1_ps = pss_pool.tile([16, 1], F32, name="a1_ps", tag="pss")
    nc.tensor.matmul(a1_ps, lhsT=ca1t, rhs=alt, start=True, stop=True)
    a1 = leaky("a1", a1_ps, 16)
    att_ps = pss_pool.tile([128, 1], F32, name="att_ps", tag="pss")
    nc.tensor.matmul(att_ps, lhsT=ca2t, rhs=a1, start=True, stop=True)
    attv = singles.tile([128, 1], F32, name="attv", tag="attv")
    nc.scalar.activation(attv, att_ps, AF.Sigmoid)

    # ---- diag weights: dg_all[:, t*128+j] = I[p,j]*ktab[p,t]; attd likewise.
    # Two halves so the first depthwise matmuls start before the second lands.
    dg_all = singles.tile([128, NT * 128], BF16, name="dg_all", tag="dg_all")
    ktab_b = ktab.unsqueeze(2).broadcast_to([128, NT, 128])
    mask3 = mask.rearrange("p (t c) -> p t c", t=NT)
    dg3 = dg_all.rearrange("p (t c) -> p t c", t=NT)
    nc.vector.tensor_tensor(
        out=dg3[:, 0:5, :], in0=mask3[:, 0:5, :], in1=ktab_b[:, 0:5, :],
        op=ALU.mult,
    )
    nc.vector.tensor_tensor(
        out=dg3[:, 5:NT, :], in0=mask3[:, 5:NT, :], in1=ktab_b[:, 5:NT, :],
        op=ALU.mult,
    )
    attd = singles.tile([128, 128], BF16, name="attd", tag="attd")
    nc.vector.tensor_tensor(
        out=attd, in0=mask[:, 0:128], in1=attv.broadcast_to([128, 128]),
        op=ALU.mult,
    )
    d["ktab"] = ktab
    d["dg_all"] = dg_all
    d["attd"] = attd


def _make_tail(nc, pso_pool, opool, xp, srelu, cwt, attd, cb, out_d, ci, tr, y0):
    """Two 4-row pointwise+residual PSUM groups, biased bf16 evacs into one
    8-row osb, single store DMA. Chunk-relative rows [tr, tr+8)."""

    def tail():
        osb = opool.tile([128, 8 * W], BF16, name=f"ob{ci}_{tr}", tag="ob")
        for h, r0 in enumerate((tr, tr + 4)):
            sl = slice(r0 * W, (r0 + 4) * W)
            pso = pso_pool.tile([128, 4 * W], F32, name=f"pso{ci}_{r0}", tag="pso")
            nc.tensor.matmul(pso, lhsT=cwt, rhs=srelu[:, sl], start=True,
                             stop=False)
            nc.tensor.matmul(
                pso, lhsT=attd, rhs=xp[:, 1 + r0 : 1 + r0 + 4, XOFF : XOFF + W],
                start=False, stop=True,
            )
            nc.scalar.activation(osb[:, h * 4 * W : (h + 1) * 4 * W], pso,
                                 AF.Identity, bias=cb)
        nc.sync.dma_start(
            out=out_d[:, (y0 + tr) * W : (y0 + tr + 8) * W], in_=osb
        )

    return tail


def build_module():
    nc = bacc.Bacc(
        "TRN2",
        target_bir_lowering=False,
        debug=False,
        enable_asserts=False,
        num_devices=B,
    )
    d = {
        "xpad": nc.dram_tensor("xpad", [C, HP * WP], BF16, kind="ExternalInput").ap(),
        "fblob": nc.dram_tensor("fblob", [128, F_COLS], F32, kind="ExternalInput").ap(),
        "bblob": nc.dram_tensor("bblob", [128, B_COLS], BF16, kind="ExternalInput").ap(),
        "out": nc.dram_tensor("out", [C, HW], BF16, kind="ExternalOutput").ap(),
    }
    with tile.TileContext(nc) as tc:
        with ExitStack() as ctx:
            d["ctx"] = ctx
            _emit(tc, nc, d)
    nc.finalize()
    return nc


_module_cache = None


def _get_module():
    global _module_cache
    if _module_cache is None:
        _module_cache = build_module()
    return _module_cache


def make_in_maps(x, altitude, W1, W2, conv_w, conv_b, ca_w1, ca_w2):
    f = np.float32
    bf = ml_dtypes.bfloat16
    x = np.asarray(x, dtype=f)
    altitude = np.asarray(altitude, dtype=f)
    xpad = np.zeros((B, C, HP, WP), dtype=f)
    xpad[:, :, 1 : H + 1, XOFF : XOFF + W] = x
    xq = np.ascontiguousarray(xpad.astype(bf).reshape(B, C, HP * WP))

    fblob_shared = np.zeros((128, F_COLS), dtype=f)
    fblob_shared[:, F_W1T : F_W1T + 128] = np.asarray(W1, dtype=f).T
    fblob_shared[:, F_CB] = np.asarray(conv_b, dtype=f)
    fblob_shared[:, F_CA1T : F_CA1T + 16] = np.asarray(ca_w1, dtype=f).T
    fblob_shared[0:16, F_CA2T : F_CA2T + 128] = np.asarray(ca_w2, dtype=f).T

    bblob = np.zeros((128, B_COLS), dtype=bf)
    bblob[:, B_W2T : B_W2T + NT * 128] = np.asarray(W2, dtype=f).T.astype(bf)
    bblob[:, B_CWT : B_CWT + 128] = np.asarray(conv_w, dtype=f).T.astype(bf)
    bblob[:, B_MASK : B_MASK + NT * 128] = np.tile(
        np.eye(128, dtype=f), (1, NT)
    ).astype(bf)
    bblob = np.ascontiguousarray(bblob)

    maps = []
    for bb in range(B):
        fblob = fblob_shared.copy()
        fblob[:, F_ALT] = altitude[bb]
        maps.append({"xpad": xq[bb], "fblob": np.ascontiguousarray(fblob),
                     "bblob": bblob})
    return maps


def kernel(x, altitude, W1, W2, conv_w, conv_b, ca_w1, ca_w2):
    global last_results
    in_maps = make_in_maps(x, altitude, W1, W2, conv_w, conv_b, ca_w1, ca_w2)
    nc = _get_module()
    trace = os.environ.get("KERNEL_TRACE", "0") == "1"
    last_results = run_bass_kernel_spmd(
        nc, in_maps, core_ids=list(range(B)), trace=trace
    )
    out = np.stack(
        [
            last_results.results[bb]["out"].astype(np.float32).reshape(C, H, W)
            for bb in range(B)
        ]
    )
    return out


# revision 6
# speedup vs baseline: 1.2284x; 1.0182x over previous
"""Trainium2 Bass kernel for nn_DA_conv: per-sample generated depthwise 3x3 conv
-> relu -> 1x1 pointwise conv (+bias) -> + x * channel_attention(altitude).

Data-parallel over batch: 8 samples -> 8 NeuronCores, weights replicated.

v4 design (trace-driven, from the 83.2us v3):
  * Depthwise split: TensorE rows [0,20) of each 32-row chunk (diag bf16
    matmuls, ~0.48us/row), VectorE rows [20,32) (tensor_scalar_mul at 4x +
    depth-4 tensor_tensor add tree at 2x, ~1.1us/row). Measured DVE op cost
    TS = 208+0.266N ns, TT = 153+0.518N ns -> balanced at 20/12.
  * Residual x*att rides the pointwise PSUM group as a diag(att) matmul.
  * All prologue matmuls bf16 (fp32 matmuls lower to LOW/HIGH pairs - 2x).
  * dg_all tensor_tensor reads ktab straight from PSUM; the SBUF ktab copy
    (needed as the DVE tap scalar) runs off the critical path.
  * Single early bf16 weight blob (w1t|alt|ca1t|ca2t|w2t|cwt) -> first
    depthwise matmul ~5us after exec start; mask blob + chunk-0 x in halves
    right behind it.
  * xb1 (odd-dx tap alignment) via +1-element-offset DMA of the padded x.
  * Output bf16 (host upcasts); ~8us fixed framework postamble and ~1.2us of
    preamble barrier are outside our control.
"""

import os
from collections import deque
from contextlib import ExitStack

import ml_dtypes
import numpy as np

import concourse.bass as bass
import concourse.mybir as mybir
import concourse.tile as tile
from concourse import bacc
from concourse.bass_utils import run_bass_kernel_spmd

AF = mybir.ActivationFunctionType
ALU = mybir.AluOpType
F32 = mybir.dt.float32
BF16 = mybir.dt.bfloat16

B, C, H, W = 8, 128, 128, 128
KK = 3
NT = KK * KK                 # 9 taps
HW = H * W
XOFF = 2                     # interior column offset in the padded layout
WP = W + 4                   # host-padded width (2 left, 2 right)
HP = H + 2                   # host-padded height (1 halo row each side)
R = 32                       # image rows per chunk
NCH = H // R                 # 4 chunks
PE_ROWS = 20                 # chunk rows [0, PE_ROWS) -> TensorE depthwise
DVE_ROWS = R - PE_ROWS       # chunk rows [PE_ROWS, R) -> VectorE depthwise
TAPS = [(dy, dx) for dy in (-1, 0, 1) for dx in (-1, 0, 1)]  # t = (dy+1)*3+(dx+1)
TAIL_LAG = 2                 # 8-row tail units kept pending (pipelining depth)

# bf16 weight blob column layout: w1t | alt | ca1t | ca2t | w2t | cwt
W_W1T, W_ALT, W_CA1T, W_CA2T = 0, 128, 129, 145
W_W2T, W_CWT = 145 + 128, 145 + 128 + NT * 128
W_COLS = W_CWT + 128

last_results = None          # BassKernelResults of the most recent run


def _emit(tc, nc, d):
    ctx = d["ctx"]
    singles = ctx.enter_context(tc.tile_pool(name="singles", bufs=1))
    xpool = ctx.enter_context(tc.tile_pool(name="xpool", bufs=3))
    xbpool = ctx.enter_context(tc.tile_pool(name="xbpool", bufs=2))
    spool = ctx.enter_context(tc.tile_pool(name="spool", bufs=2))
    tpool = ctx.enter_context(tc.tile_pool(name="tpool", bufs=12))
    opool = ctx.enter_context(tc.tile_pool(name="opool", bufs=3))
    pss_pool = ctx.enter_context(tc.tile_pool(name="psum_s", bufs=5, space="PSUM"))
    pso_pool = ctx.enter_context(tc.tile_pool(name="psum_o", bufs=3, space="PSUM"))

    wblob = singles.tile([128, W_COLS], BF16, name="wblob", tag="wblob")
    nc.sync.dma_start(out=wblob, in_=d["wblob"])
    cb = singles.tile([128, 1], F32, name="cb", tag="cb")
    nc.sync.dma_start(out=cb, in_=d["cb"])

    x3 = d["xpad"].rearrange("c (h w) -> c h w", w=WP)
    xpf_d = d["xpad"]
    NB1 = (DVE_ROWS + 2) * WP    # xb1 flat length (DVE tap rows + dy halo)

    # chunk-0 x in two halves so the first depthwise matmuls can start as
    # soon as the first 18 rows land; mask blob right behind, then the rest.
    xps, xb1s = [], []
    xp0 = xpool.tile([128, R + 2, WP], BF16, name="xp0", tag="xp")
    nc.sync.dma_start(out=xp0[:, 0:18, :], in_=x3[:, 0:18, :])
    nc.sync.dma_start(out=xp0[:, 18 : R + 2, :], in_=x3[:, 18 : R + 2, :])
    mask = singles.tile([128, NT * 128], BF16, name="mask", tag="mask")
    nc.sync.dma_start(out=mask, in_=d["mask"])
    xb1_0 = xbpool.tile([128, NB1], BF16, name="xb1_0", tag="xb1")
    nc.sync.dma_start(
        out=xb1_0[:, 0 : NB1 - 2],
        in_=xpf_d[:, PE_ROWS * WP + 1 : PE_ROWS * WP + NB1 - 1],
    )
    xps.append(xp0)
    xb1s.append(xb1_0)
    _emit_prologue(tc, nc, d, singles, pss_pool, wblob, mask)
    for ci in range(1, NCH):
        y0 = ci * R
        xp = xpool.tile([128, R + 2, WP], BF16, name=f"xp{ci}", tag="xp")
        nc.sync.dma_start(out=xp, in_=x3[:, y0 : y0 + R + 2, :])
        xb1 = xbpool.tile([128, NB1], BF16, name=f"xb1{ci}", tag="xb1")
        base = (y0 + PE_ROWS) * WP
        nc.sync.dma_start(
            out=xb1[:, 0 : NB1 - 2], in_=xpf_d[:, base + 1 : base + NB1 - 1]
        )
        xps.append(xp)
        xb1s.append(xb1)

    ktab = d["ktab"]
    dg_all = d["dg_all"]
    attd = d["attd"]
    cwt = wblob[:, W_CWT : W_CWT + 128]
    out_d = d["out"]

    tails = deque()

    def flush(n):
        while len(tails) > n:
            tails.popleft()()

    for ci in range(NCH):
        y0 = ci * R
        xp = xps[ci]
        xb13 = xb1s[ci].rearrange("p (r c) -> p r c", c=WP)
        srelu = spool.tile([128, R * W], BF16, name=f"sr{ci}", tag="sr")

        # -- DVE depthwise: rows [PE_ROWS, R): 9 products, depth-4 add tree --
        prods = []
        for ti, (dy, dx) in enumerate(TAPS):
            if dx == 0:
                src = xp[:, 1 + PE_ROWS + dy : 1 + PE_ROWS + DVE_ROWS + dy,
                         XOFF : XOFF + W]
            elif dx == 1:
                src = xb13[:, 1 + dy : 1 + DVE_ROWS + dy, XOFF : XOFF + W]
            else:
                src = xb13[:, 1 + dy : 1 + DVE_ROWS + dy, 0:W]
            t = tpool.tile([128, DVE_ROWS * W], BF16, name=f"tp{ci}_{ti}", tag="tp")
            nc.vector.tensor_scalar_mul(out=t, in0=src, scalar1=ktab[:, ti : ti + 1])
            prods.append(t)
        while len(prods) > 1:
            nxt = []
            for i in range(0, len(prods) - 1, 2):
                a, b = prods[i], prods[i + 1]
                nc.vector.tensor_tensor(out=a, in0=b, in1=a, op=ALU.add)
                nxt.append(a)
            if len(prods) % 2:
                nxt.append(prods[-1])
            prods = nxt
        nc.scalar.activation(srelu[:, PE_ROWS * W : R * W], prods[0], AF.Relu)

        # -- PE depthwise: rows [0, PE_ROWS) in 4-row PSUM blocks --
        for rs in range(0, PE_ROWS, 4):
            pss = pss_pool.tile([128, 4 * W], F32, name=f"pss{ci}_{rs}", tag="pss")
            for ti, (dy, dx) in enumerate(TAPS):
                rhs = xp[:, 1 + rs + dy : 5 + rs + dy, XOFF + dx : XOFF + dx + W]
                nc.tensor.matmul(
                    pss, lhsT=dg_all[:, ti * 128 : (ti + 1) * 128], rhs=rhs,
                    start=(ti == 0), stop=(ti == NT - 1),
                )
            nc.scalar.activation(srelu[:, rs * W : (rs + 4) * W], pss, AF.Relu)

        # -- tails: 8-row units (2 PSUM banks of pointwise+residual, 1 store) --
        lag = 0 if ci == NCH - 1 else TAIL_LAG
        for tr in range(0, R, 8):
            tails.append(_make_tail(nc, pso_pool, opool, xp, srelu, cwt, attd,
                                    cb, out_d, ci, tr, y0))
            flush(lag)
    flush(0)


def _emit_prologue(tc, nc, d, singles, pss_pool, wblob, mask):
    alt = wblob[:, W_ALT : W_ALT + 1]
    w1t = wblob[:, W_W1T : W_W1T + 128]
    ca1t = wblob[:, W_CA1T : W_CA1T + 16]
    ca2t = wblob[0:16, W_CA2T : W_CA2T + 128]
    w2t = wblob[:, W_W2T : W_W2T + NT * 128]

    def leaky(name, psum_src, parts, dt=F32):
        """lrelu(v) = max(0.1*v, v), via ACT copy to SBUF then one DVE STT."""
        tmp = singles.tile([parts, 1], F32, name=f"{name}_t", tag=f"{name}_t")
        nc.scalar.activation(tmp, psum_src, AF.Copy)
        res = singles.tile([parts, 1], dt, name=name, tag=name)
        nc.vector.scalar_tensor_tensor(
            out=res, in0=tmp, scalar=0.1, in1=tmp, op0=ALU.mult, op1=ALU.max
        )
        return res

    # ---- kernel-generator MLP (all bf16 matmuls) ----
    feat_ps = pss_pool.tile([128, 1], F32, name="feat_ps", tag="pss")
    nc.tensor.matmul(feat_ps, lhsT=w1t, rhs=alt, start=True, stop=True)
    feat = leaky("feat", feat_ps, 128, dt=BF16)

    ktab_ps = pss_pool.tile([128, NT], F32, name="ktab_ps", tag="pss")
    w2r = w2t.rearrange("p (c t) -> p t c", t=NT)
    for t in range(NT):
        nc.tensor.matmul(
            ktab_ps[:, t : t + 1], lhsT=w2r[:, t, :], rhs=feat, start=True, stop=True
        )
    # SBUF copy (DVE tap scalar source) runs in parallel with the dg build.
    ktab = singles.tile([128, NT], F32, name="ktab", tag="ktab")
    nc.scalar.activation(ktab, ktab_ps, AF.Copy)

    # ---- diag weights straight from PSUM: dg[:, t*128+j] = I[p,j]*ktab[p,t]
    dg_all = singles.tile([128, NT * 128], BF16, name="dg_all", tag="dg_all")
    ktab_b = ktab_ps.unsqueeze(2).broadcast_to([128, NT, 128])
    mask3 = mask.rearrange("p (t c) -> p t c", t=NT)
    dg3 = dg_all.rearrange("p (t c) -> p t c", t=NT)
    nc.vector.tensor_tensor(
        out=dg3[:, 0:5, :], in0=mask3[:, 0:5, :], in1=ktab_b[:, 0:5, :],
        op=ALU.mult,
    )
    nc.vector.tensor_tensor(
        out=dg3[:, 5:NT, :], in0=mask3[:, 5:NT, :], in1=ktab_b[:, 5:NT, :],
        op=ALU.mult,
    )

    # ---- channel attention (bf16 matmuls) ----
    a1_ps = pss_pool.tile([16, 1], F32, name="a1_ps", tag="pss")
    nc.tensor.matmul(a1_ps, lhsT=ca1t, rhs=alt, start=True, stop=True)
    a1 = leaky("a1", a1_ps, 16, dt=BF16)
    att_ps = pss_pool.tile([128, 1], F32, name="att_ps", tag="pss")
    nc.tensor.matmul(att_ps, lhsT=ca2t, rhs=a1, start=True, stop=True)
    attv = singles.tile([128, 1], F32, name="attv", tag="attv")
    nc.scalar.activation(attv, att_ps, AF.Sigmoid)
    attd = singles.tile([128, 128], BF16, name="attd", tag="attd")
    nc.vector.tensor_tensor(
        out=attd, in0=mask[:, 0:128], in1=attv.broadcast_to([128, 128]),
        op=ALU.mult,
    )
    d["ktab"] = ktab
    d["dg_all"] = dg_all
    d["attd"] = attd


def _make_tail(nc, pso_pool, opool, xp, srelu, cwt, attd, cb, out_d, ci, tr, y0):
    """Two 4-row pointwise+residual PSUM groups, biased bf16 evacs into one
    8-row osb, single store DMA. Chunk-relative rows [tr, tr+8)."""

    def tail():
        osb = opool.tile([128, 8 * W], BF16, name=f"ob{ci}_{tr}", tag="ob")
        for h, r0 in enumerate((tr, tr + 4)):
            sl = slice(r0 * W, (r0 + 4) * W)
            pso = pso_pool.tile([128, 4 * W], F32, name=f"pso{ci}_{r0}", tag="pso")
            nc.tensor.matmul(pso, lhsT=cwt, rhs=srelu[:, sl], start=True,
                             stop=False)
            nc.tensor.matmul(
                pso, lhsT=attd, rhs=xp[:, 1 + r0 : 1 + r0 + 4, XOFF : XOFF + W],
                start=False, stop=True,
            )
            nc.scalar.activation(osb[:, h * 4 * W : (h + 1) * 4 * W], pso,
                                 AF.Identity, bias=cb)
        nc.sync.dma_start(
            out=out_d[:, (y0 + tr) * W : (y0 + tr + 8) * W], in_=osb
        )

    return tail


def build_module():
    nc = bacc.Bacc(
        "TRN2",
        target_bir_lowering=False,
        debug=False,
        enable_asserts=False,
        num_devices=B,
    )
    d = {
        "xpad": nc.dram_tensor("xpad", [C, HP * WP], BF16, kind="ExternalInput").ap(),
        "wblob": nc.dram_tensor("wblob", [128, W_COLS], BF16, kind="ExternalInput").ap(),
        "cb": nc.dram_tensor("cb", [C, 1], F32, kind="ExternalInput").ap(),
        "mask": nc.dram_tensor("mask", [128, NT * 128], BF16, kind="ExternalInput").ap(),
        "out": nc.dram_tensor("out", [C, HW], BF16, kind="ExternalOutput").ap(),
    }
    with tile.TileContext(nc) as tc:
        with ExitStack() as ctx:
            d["ctx"] = ctx
            _emit(tc, nc, d)
    nc.finalize()
    return nc


_module_cache = None


def _get_module():
    global _module_cache
    if _module_cache is None:
        _module_cache = build_module()
    return _module_cache


def make_in_maps(x, altitude, W1, W2, conv_w, conv_b, ca_w1, ca_w2):
    f = np.float32
    bf = ml_dtypes.bfloat16
    x = np.asarray(x, dtype=f)
    altitude = np.asarray(altitude, dtype=f)
    xpad = np.zeros((B, C, HP, WP), dtype=f)
    xpad[:, :, 1 : H + 1, XOFF : XOFF + W] = x
    xq = np.ascontiguousarray(xpad.astype(bf).reshape(B, C, HP * WP))

    wblob_shared = np.zeros((128, W_COLS), dtype=bf)
    wblob_shared[:, W_W1T : W_W1T + 128] = np.asarray(W1, dtype=f).T.astype(bf)
    wblob_shared[:, W_CA1T : W_CA1T + 16] = np.asarray(ca_w1, dtype=f).T.astype(bf)
    wblob_shared[0:16, W_CA2T : W_CA2T + 128] = np.asarray(
        ca_w2, dtype=f
    ).T.astype(bf)
    wblob_shared[:, W_W2T : W_W2T + NT * 128] = np.asarray(
        W2, dtype=f
    ).T.astype(bf)
    wblob_shared[:, W_CWT : W_CWT + 128] = np.asarray(conv_w, dtype=f).T.astype(bf)

    cb_arr = np.ascontiguousarray(np.asarray(conv_b, dtype=f).reshape(C, 1))
    mask_arr = np.ascontiguousarray(
        np.tile(np.eye(128, dtype=f), (1, NT)).astype(bf)
    )

    maps = []
    for bb in range(B):
        wblob = wblob_shared.copy()
        wblob[:, W_ALT] = altitude[bb].astype(bf)
        maps.append({"xpad": xq[bb], "wblob": np.ascontiguousarray(wblob),
                     "cb": cb_arr, "mask": mask_arr})
    return maps


def kernel(x, altitude, W1, W2, conv_w, conv_b, ca_w1, ca_w2):
    global last_results
    in_maps = make_in_maps(x, altitude, W1, W2, conv_w, conv_b, ca_w1, ca_w2)
    nc = _get_module()
    trace = os.environ.get("KERNEL_TRACE", "0") == "1"
    last_results = run_bass_kernel_spmd(
        nc, in_maps, core_ids=list(range(B)), trace=trace
    )
    out = np.stack(
        [
            last_results.results[bb]["out"].astype(np.float32).reshape(C, H, W)
            for bb in range(B)
        ]
    )
    return out
